# revision 5
# baseline (speedup 1.0000x reference)
"""AdaBIGGAN adaptive 1x1-conv stage, data-parallel across 8 TRN2 NeuronCores.

Math (per sample b):
    scale[b, c] = sum_k y[b, k] * Wsum[c, k] + bsum[c]
        where Wsum[c, k] = sum_j Wg_w[c*C + j, k],  bsum[c] = sum_j Wg_b[c*C + j]
    bias[b, c]  = sum_k y[b, k] * Bg_w[c, k] + Bg_b[c]
    out[b, c, :, :] = relu(h[b, c, :, :] * scale[b, c] + bias[b, c])

Sharding: batch B=32 split 4-per-core across 8 cores; hypernet params replicated.
"""

import numpy as np

import concourse.bacc as bacc
import concourse.mybir as mybir
from concourse.tile import TileContext
from concourse.bass_utils import run_bass_kernel_spmd

_B, _C, _H, _W, _IN = 32, 96, 128, 128, 148
_NCORES = 8
_BL = _B // _NCORES          # 4 samples per core
_HW = _H * _W                # 16384
_FCH = 4096                  # free-dim chunk of the h stream
_F32 = mybir.dt.float32

# Results of the last run_bass_kernel_spmd call (exec_time_ns etc) for test harness.
LAST_RESULTS = None


def _build():
    nc = bacc.Bacc(None)
    h = nc.declare_dram_parameter("h", [_BL * _C, _HW], _F32, isOutput=False)
    yb = nc.declare_dram_parameter("yb", [_C, _BL * _IN], _F32, isOutput=False)
    wg = nc.declare_dram_parameter("wg", [_C, _C * _IN], _F32, isOutput=False)
    wb = nc.declare_dram_parameter("wb", [_C, _C], _F32, isOutput=False)
    bw = nc.declare_dram_parameter("bw", [_C, _IN], _F32, isOutput=False)
    bb = nc.declare_dram_parameter("bb", [_C, 1], _F32, isOutput=False)
    out = nc.declare_dram_parameter("out", [_BL * _C, _HW], _F32, isOutput=True)

    with TileContext(nc) as tc:
        with (
            tc.tile_pool(name="hyper", bufs=1) as hp,
            tc.tile_pool(name="stream", bufs=6) as sp,
        ):
            # --- hypernetwork: per-(b,c) scale/bias scalars -------------------
            wg_t = hp.tile([_C, _C * _IN], _F32)   # [c, (j k)]
            nc.sync.dma_start(out=wg_t[:], in_=wg[:])
            wb_t = hp.tile([_C, _C], _F32)         # [c, j]
            nc.sync.dma_start(out=wb_t[:], in_=wb[:])
            bw_t = hp.tile([_C, _IN], _F32)        # [c, k]
            nc.sync.dma_start(out=bw_t[:], in_=bw[:])
            bb_t = hp.tile([_C, 1], _F32)          # [c]
            nc.sync.dma_start(out=bb_t[:], in_=bb[:])
            y_t = hp.tile([_C, _BL * _IN], _F32)   # y broadcast to all c rows
            nc.sync.dma_start(out=y_t[:], in_=yb[:])

            # Wsum[c, k] = sum_j Wg_w[(c j), k]: reduce j (strided innermost view)
            wsum = hp.tile([_C, _IN], _F32)
            nc.vector.tensor_reduce(
                out=wsum[:],
                in_=wg_t[:].rearrange("c (j k) -> c k j", j=_C, k=_IN),
                axis=mybir.AxisListType.X,
                op=mybir.AluOpType.add,
            )
            # bsum[c] = sum_j Wg_b[(c j)]
            bsum = hp.tile([_C, 1], _F32)
            nc.vector.tensor_reduce(
                out=bsum[:], in_=wb_t[:],
                axis=mybir.AxisListType.X, op=mybir.AluOpType.add,
            )

            scale_t = hp.tile([_C, _BL], _F32)     # scale^T: [c, b]
            bias_t = hp.tile([_C, _BL], _F32)      # bias^T:  [c, b]
            junk = hp.tile([_C, _IN], _F32)
            junk2 = hp.tile([_C, _IN], _F32)
            for b in range(_BL):
                yb_ap = y_t[:, b * _IN:(b + 1) * _IN]
                # scale^T[:, b] = sum_k(Wsum * y_b)
                nc.vector.tensor_mul(junk[:], wsum[:], yb_ap)
                nc.vector.tensor_reduce(
                    out=scale_t[:, b:b + 1], in_=junk[:],
                    axis=mybir.AxisListType.X, op=mybir.AluOpType.add,
                )
                # bias^T[:, b] = sum_k(Bg_w * y_b)
                nc.vector.tensor_mul(junk2[:], bw_t[:], yb_ap)
                nc.vector.tensor_reduce(
                    out=bias_t[:, b:b + 1], in_=junk2[:],
                    axis=mybir.AxisListType.X, op=mybir.AluOpType.add,
                )
            nc.vector.tensor_scalar_add(scale_t[:], scale_t[:], bsum[:])
            nc.vector.tensor_scalar_add(bias_t[:], bias_t[:], bb_t[:])

            # --- stream h: out = relu(h * scale + bias), fused in ScalarE ----
            for b in range(_BL):
                rows = slice(b * _C, (b + 1) * _C)
                for f0 in range(0, _HW, _FCH):
                    t = sp.tile([_C, _FCH], _F32)
                    nc.sync.dma_start(out=t[:], in_=h[rows, f0:f0 + _FCH])
                    nc.scalar.activation(
                        out=t[:], in_=t[:],
                        func=mybir.ActivationFunctionType.Relu,
                        bias=bias_t[:, b:b + 1],
                        scale=scale_t[:, b:b + 1],
                    )
                    nc.sync.dma_start(out=out[rows, f0:f0 + _FCH], in_=t[:])
    nc.finalize()
    return nc


def kernel(h, y, Wg_w, Wg_b, Bg_w, Bg_b):
    global LAST_RESULTS
    h = np.ascontiguousarray(h, np.float32)
    y = np.ascontiguousarray(y, np.float32)

    nc = _build()
    wg_r = np.ascontiguousarray(Wg_w, np.float32).reshape(_C, _C * _IN)
    wb_r = np.ascontiguousarray(Wg_b, np.float32).reshape(_C, _C)
    bw_r = np.ascontiguousarray(Bg_w, np.float32)
    bb_r = np.ascontiguousarray(Bg_b, np.float32).reshape(_C, 1)

    in_maps = []
    for i in range(_NCORES):
        hs = h[i * _BL:(i + 1) * _BL].reshape(_BL * _C, _HW)
        ys = y[i * _BL:(i + 1) * _BL].reshape(1, _BL * _IN)
        in_maps.append({
            "h": np.ascontiguousarray(hs),
            "yb": np.ascontiguousarray(np.broadcast_to(ys, (_C, _BL * _IN))),
            "wg": wg_r, "wb": wb_r, "bw": bw_r, "bb": bb_r,
        })

    res = run_bass_kernel_spmd(nc, in_maps, core_ids=list(range(_NCORES)))
    LAST_RESULTS = res
    outs = [r["out"].reshape(_BL, _C, _H, _W) for r in res.results]
    return np.concatenate(outs, axis=0)


# revision 6
# speedup vs baseline: 1.4323x; 1.4323x over previous
"""AdaBIGGAN adaptive 1x1-conv stage, data-parallel across 8 TRN2 NeuronCores.

Math (per sample b):
    scale[b, c] = sum_k y[b, k] * Wsum[c, k] + bsum[c]
        where Wsum[c, k] = sum_j Wg_w[c*C + j, k],  bsum[c] = sum_j Wg_b[c*C + j]
    bias[b, c]  = sum_k y[b, k] * Bg_w[c, k] + Bg_b[c]
    out[b, c, :, :] = relu(h[b, c, :, :] * scale[b, c] + bias[b, c])

Sharding: batch B=32 split 4-per-core across 8 cores; hypernet params replicated.
"""

import numpy as np

import concourse.bacc as bacc
import concourse.mybir as mybir
from concourse.tile import TileContext
from concourse.bass_utils import run_bass_kernel_spmd

_B, _C, _H, _W, _IN = 32, 96, 128, 128, 148
_NCORES = 8
_BL = _B // _NCORES          # 4 samples per core
_HW = _H * _W                # 16384
_ROWS = _BL * _C             # 384 rows = 3 x 128 partitions
_NPT = 3                     # row tiles of 128
_FCH = 4096                  # free-dim chunk of the h stream
_F32 = mybir.dt.float32

LAST_RESULTS = None


def _build():
    nc = bacc.Bacc(None)
    h = nc.declare_dram_parameter("h", [_ROWS, _HW], _F32, isOutput=False)
    yb = nc.declare_dram_parameter("yb", [_C, _BL * _IN], _F32, isOutput=False)
    wg = nc.declare_dram_parameter("wg", [_C, _C * _IN], _F32, isOutput=False)
    wb = nc.declare_dram_parameter("wb", [_C, _C], _F32, isOutput=False)
    bw = nc.declare_dram_parameter("bw", [_C, _IN], _F32, isOutput=False)
    bb = nc.declare_dram_parameter("bb", [_C, 1], _F32, isOutput=False)
    out = nc.declare_dram_parameter("out", [_ROWS, _HW], _F32, isOutput=True)

    with TileContext(nc) as tc:
        with (
            tc.tile_pool(name="hyper", bufs=1) as hp,
            tc.tile_pool(name="stream", bufs=6) as sp,
            tc.tile_pool(name="scratch", bufs=1, space="DRAM") as dp,
        ):
            # --- hypernetwork: per-(b,c) scale/bias scalars -------------------
            # all hyper loads ride the gpsimd SWDGE queue so they don't
            # head-of-line-block the h stream on the sync HWDGE ring
            wg_t = hp.tile([_C, _C * _IN], _F32)   # [c, (j k)] j-major
            nc.gpsimd.dma_start(out=wg_t[:], in_=wg[:])
            wb_t = hp.tile([_C, _C], _F32)         # [c, j]
            nc.gpsimd.dma_start(out=wb_t[:], in_=wb[:])
            bw_t = hp.tile([_C, _IN], _F32)        # [c, k]
            nc.gpsimd.dma_start(out=bw_t[:], in_=bw[:])
            bb_t = hp.tile([_C, 1], _F32)          # [c]
            nc.gpsimd.dma_start(out=bb_t[:], in_=bb[:])
            y_t = hp.tile([_C, _BL * _IN], _F32)   # y broadcast to all c rows
            nc.gpsimd.dma_start(out=y_t[:], in_=yb[:])

            # Wsum[c, k] = sum_j Wg_w[(c j), k]: tree-halve j (contiguous adds,
            # DVE 2x mode) 96 -> 48 -> 24 -> 12 -> 6 -> 3, then strided reduce.
            j = _C
            while j > 3:
                half = j // 2 * _IN
                nc.vector.tensor_add(wg_t[:, :half], wg_t[:, :half],
                                     wg_t[:, half:2 * half])
                j //= 2
            wsum = hp.tile([_C, _IN], _F32)
            nc.vector.tensor_reduce(
                out=wsum[:],
                in_=wg_t[:, :3 * _IN].rearrange("c (j k) -> c k j", j=3, k=_IN),
                axis=mybir.AxisListType.X,
                op=mybir.AluOpType.add,
            )
            # bsum[c] = sum_j Wg_b[(c j)]
            bsum = hp.tile([_C, 1], _F32)
            nc.vector.tensor_reduce(
                out=bsum[:], in_=wb_t[:],
                axis=mybir.AxisListType.X, op=mybir.AluOpType.add,
            )

            scale_t = hp.tile([_C, _BL], _F32)     # scale^T: [c, b]
            bias_t = hp.tile([_C, _BL], _F32)      # bias^T:  [c, b]
            junk = hp.tile([_C, _IN], _F32)
            junk2 = hp.tile([_C, _IN], _F32)
            for b in range(_BL):
                yb_ap = y_t[:, b * _IN:(b + 1) * _IN]
                nc.vector.tensor_mul(junk[:], wsum[:], yb_ap)
                nc.vector.tensor_reduce(
                    out=scale_t[:, b:b + 1], in_=junk[:],
                    axis=mybir.AxisListType.X, op=mybir.AluOpType.add,
                )
                nc.vector.tensor_mul(junk2[:], bw_t[:], yb_ap)
                nc.vector.tensor_reduce(
                    out=bias_t[:, b:b + 1], in_=junk2[:],
                    axis=mybir.AxisListType.X, op=mybir.AluOpType.add,
                )
            nc.vector.tensor_scalar_add(scale_t[:], scale_t[:], bsum[:])
            nc.vector.tensor_scalar_add(bias_t[:], bias_t[:], bb_t[:])

            # Re-lay [c, b] -> flat [b*C + c] via DRAM bounce so stream tiles
            # can use all 128 partitions.
            sb_scr = dp.tile([2, _ROWS], _F32)
            for b in range(_BL):
                nc.gpsimd.dma_start(out=sb_scr[0, b * _C:(b + 1) * _C],
                                    in_=scale_t[:, b:b + 1])
                nc.gpsimd.dma_start(out=sb_scr[1, b * _C:(b + 1) * _C],
                                    in_=bias_t[:, b:b + 1])
            scale_fl = []
            bias_fl = []
            for r in range(_NPT):
                sf = hp.tile([128, 1], _F32, tag=f"sf{r}")
                nc.gpsimd.dma_start(out=sf[:], in_=sb_scr[0, r * 128:(r + 1) * 128])
                bf = hp.tile([128, 1], _F32, tag=f"bf{r}")
                nc.gpsimd.dma_start(out=bf[:], in_=sb_scr[1, r * 128:(r + 1) * 128])
                scale_fl.append(sf)
                bias_fl.append(bf)

            # --- stream h: out = relu(h * scale + bias), fused in ScalarE ----
            # loads on sync HWDGE ring, stores on scalar HWDGE ring
            for r in range(_NPT):
                rows = slice(r * 128, (r + 1) * 128)
                for f0 in range(0, _HW, _FCH):
                    t = sp.tile([128, _FCH], _F32)
                    nc.sync.dma_start(out=t[:], in_=h[rows, f0:f0 + _FCH])
                    nc.scalar.activation(
                        out=t[:], in_=t[:],
                        func=mybir.ActivationFunctionType.Relu,
                        bias=bias_fl[r][:],
                        scale=scale_fl[r][:],
                    )
                    nc.scalar.dma_start(out=out[rows, f0:f0 + _FCH], in_=t[:])
    nc.finalize()
    return nc


def kernel(h, y, Wg_w, Wg_b, Bg_w, Bg_b):
    global LAST_RESULTS
    h = np.ascontiguousarray(h, np.float32)
    y = np.ascontiguousarray(y, np.float32)

    nc = _build()
    wg_r = np.ascontiguousarray(Wg_w, np.float32).reshape(_C, _C * _IN)
    wb_r = np.ascontiguousarray(Wg_b, np.float32).reshape(_C, _C)
    bw_r = np.ascontiguousarray(Bg_w, np.float32)
    bb_r = np.ascontiguousarray(Bg_b, np.float32).reshape(_C, 1)

    in_maps = []
    for i in range(_NCORES):
        hs = h[i * _BL:(i + 1) * _BL].reshape(_ROWS, _HW)
        ys = y[i * _BL:(i + 1) * _BL].reshape(1, _BL * _IN)
        in_maps.append({
            "h": np.ascontiguousarray(hs),
            "yb": np.ascontiguousarray(np.broadcast_to(ys, (_C, _BL * _IN))),
            "wg": wg_r, "wb": wb_r, "bw": bw_r, "bb": bb_r,
        })

    res = run_bass_kernel_spmd(nc, in_maps, core_ids=list(range(_NCORES)))
    LAST_RESULTS = res
    outs = [r["out"].reshape(_BL, _C, _H, _W) for r in res.results]
    return np.concatenate(outs, axis=0)
